# revision 1
# baseline (speedup 1.0000x reference)
"""BiLSTM-CRF loss kernel for 8x Trainium2 NeuronCores (Bass/Tile).

Sharding: data-parallel over batch (16 sentences per core). Each core runs the
identical SPMD program: embedding gather -> 2 BiLSTM layers (fwd+bwd scans
interleaved per tick) -> emissions -> CRF forward algorithm (exp-space with
periodic rescaling) + path-score numerator. Host sums the per-core partials.

Math notes (per-core, b=16, batch rows on partitions):
 - tanh(x) = 2*sigmoid(2x) - 1 everywhere, so one Sigmoid activation covers all
   four gates.  With h~ = h/2 and c~ = c/2:
     s = sigmoid(z'), z' row-scaled so s_g = sigmoid(2 z_g)
     u  = (s_g - 0.5) * s_i          ( = i*g/2 )
     c~ = s_f * c~_prev + u
     h~ = (sigmoid(4 c~) - 0.5) * s_o
   The factors of 2 are folded into the weights host-side.
 - CRF forward pass runs in exp space:  a_t = (Eexp^T a_{t-1}) .* exp(em_t),
   Eexp = exp(trans), with a partition-sum rescale every RESCALE steps whose
   log is accumulated.  logZ = ln(sum_j a_T exp(end_j)) + sum(ln rescales).
"""

import sys

sys.path.insert(0, "/opt/trn_rl_repo")

import contextlib

import numpy as np
import ml_dtypes

import concourse.bass as bass
import concourse.tile as tile
from concourse import bacc, mybir
from concourse.masks import make_identity
from concourse.bass_utils import run_bass_kernel_spmd

F32 = mybir.dt.float32
F32R = mybir.dt.float32r
BF16 = mybir.dt.bfloat16
I16 = mybir.dt.int16
AF = mybir.ActivationFunctionType
OP = mybir.AluOpType

NCORES = 8
B, T, E, H, K, V = 128, 512, 128, 128, 20, 30000
G4 = 4 * H          # 512
BL = B // NCORES    # 16 sentences per core
RESCALE = 8


def _mm(nc, out, lhsT, rhs, start, stop, fast=True):
    nc.tensor.matmul(out, lhsT, rhs, start=start, stop=stop)


def build(nt=T):
    """Build the SPMD program for sequence length nt (nt=T for real use)."""
    nc = bacc.Bacc("TRN2", target_bir_lowering=False, debug=False,
                   num_devices=NCORES)
    NTB = nt * BL   # flattened (t,b) count per core

    # ---- DRAM I/O ----
    embedb = nc.dram_tensor("embedb", [V, E], BF16, kind="ExternalInput")
    toks16 = nc.dram_tensor("toks16", [BL, nt], I16, kind="ExternalInput")
    tagsf = nc.dram_tensor("tagsf", [1, NTB], F32, kind="ExternalInput")  # b-major
    wihT0 = nc.dram_tensor("wihT0", [2, E, G4], F32R, kind="ExternalInput")
    whhT0 = nc.dram_tensor("whhT0", [2, H, G4], F32R, kind="ExternalInput")
    b0v = nc.dram_tensor("b0v", [2, 1, G4], F32R, kind="ExternalInput")
    wih1T = nc.dram_tensor("wih1T", [2, 2, H, G4], F32R, kind="ExternalInput")
    whh1T = nc.dram_tensor("whh1T", [2, H, G4], F32R, kind="ExternalInput")
    b1v = nc.dram_tensor("b1v", [2, 1, G4], F32R, kind="ExternalInput")
    woutT = nc.dram_tensor("woutT", [2, H, K], F32R, kind="ExternalInput")
    boutv = nc.dram_tensor("boutv", [K, 1], F32, kind="ExternalInput")
    transm = nc.dram_tensor("transm", [K, K], F32, kind="ExternalInput")
    startv = nc.dram_tensor("startv", [K, 1], F32, kind="ExternalInput")
    endv = nc.dram_tensor("endv", [K, 1], F32, kind="ExternalInput")
    outm = nc.dram_tensor("outm", [2, BL], F32, kind="ExternalOutput")

    with tile.TileContext(nc) as tc, contextlib.ExitStack() as ctx:
        big = ctx.enter_context(tc.tile_pool(name="big", bufs=1))
        wp = ctx.enter_context(tc.tile_pool(name="wp", bufs=1))
        work = ctx.enter_context(tc.tile_pool(name="work", bufs=3))
        stp = ctx.enter_context(tc.tile_pool(name="stp", bufs=2))

        # ---------------- P0: constants, weights, gather ----------------
        idx = wp.tile([128, nt], I16, tag="idx")
        nc.gpsimd.memset(idx[:], 0)
        nc.sync.dma_start(out=idx[0:BL, :], in_=toks16[:, :])

        def load_w(name, dram_ap, shape, dt=F32):
            t = wp.tile(shape, dt, tag=name)
            nc.sync.dma_start(out=t[:], in_=dram_ap)
            return t

        wih0_sb = [load_w(f"wih0_{d}", wihT0[d], [E, G4], F32R) for d in range(2)]
        whh0_sb = [load_w(f"whh0_{d}", whhT0[d], [H, G4], F32R) for d in range(2)]
        b0_sb = [load_w(f"b0_{d}", b0v[d], [1, G4], F32R) for d in range(2)]
        wih1_sb = [[load_w(f"wih1_{d}{h}", wih1T[d, h], [H, G4], F32R)
                    for h in range(2)] for d in range(2)]
        whh1_sb = [load_w(f"whh1_{d}", whh1T[d], [H, G4], F32R) for d in range(2)]
        b1_sb = [load_w(f"b1_{d}", b1v[d], [1, G4], F32R) for d in range(2)]
        wout_sb = [load_w(f"wout_{d}", woutT[d], [H, K], F32R) for d in range(2)]
        bout_sb = load_w("bout", boutv[:, :], [K, 1])
        trans_sb = load_w("trans", transm[:, :], [K, K])
        start_sb = load_w("start", startv[:, :], [K, 1])
        end_sb = load_w("end", endv[:, :], [K, 1])

        id16 = wp.tile([BL, BL], F32, tag="id16")
        make_identity(nc, id16[:])
        ones16f = wp.tile([1, BL], F32, tag="ones16f")
        nc.vector.memset(ones16f[:], 1.0)
        ones16 = wp.tile([1, BL], F32R, tag="ones16")
        nc.vector.tensor_copy(ones16[:], ones16f[:])
        ones20 = wp.tile([K, 1], F32, tag="ones20")
        nc.vector.memset(ones20[:], 1.0)
        ones2020 = wp.tile([K, K], F32, tag="ones2020")
        nc.vector.memset(ones2020[:], 1.0)
        iota20 = wp.tile([K, 1], mybir.dt.int32, tag="iota20i")
        nc.gpsimd.iota(iota20[:], pattern=[[0, 1]], base=0,
                       channel_multiplier=1)
        iota20f = wp.tile([K, 1], F32, tag="iota20f")
        nc.vector.tensor_copy(iota20f[:], iota20[:])
        eexp = wp.tile([K, K], F32, tag="eexp")
        nc.scalar.activation(eexp[:], trans_sb[:], AF.Exp)
        expstart = wp.tile([K, 1], F32, tag="expstart")
        nc.scalar.activation(expstart[:], start_sb[:], AF.Exp)
        expend = wp.tile([K, 1], F32, tag="expend")
        nc.scalar.activation(expend[:], end_sb[:], AF.Exp)

        # Embedding gather (+transpose): xg[128_E, NTB] bf16, col = t*BL+b
        import os
        xg = big.tile([128, 1, NTB], BF16, tag="bigB")
        if os.environ.get("KK_NO_GATHER"):
            nc.gpsimd.memset(xg[:], 0)
        else:
            GCH = 256  # idxs per gather (SWDGE descriptor-ring limit)
            for g in range(max(1, NTB // GCH)):
                cw = min(GCH, NTB)
                nc.gpsimd.dma_gather(
                    xg[:, :, g * cw:(g + 1) * cw], embedb[:, :],
                    idx[:, g * (cw // 16):(g + 1) * (cw // 16)],
                    cw, cw, E, transpose=True)
        xT = big.tile([128, NTB], F32R, tag="bigA")
        nc.vector.tensor_copy(xT[:], xg[:, 0, :])

        # Histories (feature-on-partition, t-major slices of width BL)
        h0T = [big.tile([H, NTB], F32R, tag=f"h0T{d}", name=f"h0T{d}")
               for d in range(2)]

        # ---------------- P1 / P2: the two BiLSTM layers ----------------
        def scan_layer(layer, hist_out):
            """One BiLSTM layer: fwd+bwd scans as two independent chains."""
            with tc.tile_pool(name=f"zp{layer}", bufs=2, space="PSUM") as zp, \
                 tc.tile_pool(name=f"tp{layer}", bufs=2, space="PSUM") as tp:
                cprev = []
                for d in range(2):
                    cp0 = stp.tile([BL, H], F32, tag=f"c{layer}{d}",
                                   name=f"c{layer}{d}")
                    nc.vector.memset(cp0[:], 0.0)
                    cprev.append(cp0)
                for n in range(nt):
                    tt = [n, nt - 1 - n]     # [fwd t, bwd t]
                    for d in range(2):
                        t_ = tt[d]
                        sl = slice(t_ * BL, (t_ + 1) * BL)
                        z = zp.tile([BL, G4], F32, tag=f"z{d}", name=f"z{d}")
                        if layer == 0:
                            _mm(nc, z[:], xT[:, sl], wih0_sb[d][:],
                                start=True, stop=False)
                        else:
                            _mm(nc, z[:], h0T[0][:, sl], wih1_sb[d][0][:],
                                start=True, stop=False)
                            _mm(nc, z[:], h0T[1][:, sl], wih1_sb[d][1][:],
                                start=False, stop=False)
                        wb = (whh0_sb, b0_sb) if layer == 0 else \
                             (whh1_sb, b1_sb)
                        _mm(nc, z[:], ones16[:], wb[1][d][:],
                            start=False, stop=(n == 0))
                        if n > 0:
                            tprev = tt[d] + (-1 if d == 0 else 1)
                            psl = slice(tprev * BL, (tprev + 1) * BL)
                            _mm(nc, z[:], hist_out[d][:, psl], wb[0][d][:],
                                start=False, stop=True)
                        # gates (one chain per direction); i,f,g sigmoid is
                        # on the critical path, o-gate sigmoid is not.
                        s = work.tile([BL, G4], F32, tag=f"s{d}",
                                      name=f"s{d}")
                        nc.scalar.activation(s[:], z[:], AF.Sigmoid)
                        si = s[:, 0 * H:1 * H]
                        sf = s[:, 1 * H:2 * H]
                        sg = s[:, 2 * H:3 * H]
                        so = s[:, 3 * H:4 * H]
                        u = work.tile([BL, H], F32, tag=f"u{d}", name=f"u{d}")
                        nc.vector.scalar_tensor_tensor(
                            u[:], sg, -0.5, si, OP.add, OP.mult)
                        fc = work.tile([BL, H], F32, tag=f"fc{d}",
                                       name=f"fc{d}")
                        nc.vector.tensor_tensor(fc[:], sf, cprev[d][:],
                                                OP.mult)
                        cnew = stp.tile([BL, H], F32, tag=f"c{layer}{d}",
                                        name=f"cn{layer}{d}")
                        nc.vector.tensor_tensor(cnew[:], fc[:], u[:], OP.add)
                        sc = work.tile([BL, H], F32, tag=f"sc{d}",
                                       name=f"sc{d}")
                        nc.scalar.activation(sc[:], cnew[:], AF.Sigmoid,
                                             scale=4.0)
                        hb = work.tile([BL, H], F32, tag=f"hb{d}",
                                       name=f"hb{d}")
                        nc.vector.scalar_tensor_tensor(
                            hb[:], sc[:], -0.5, so, OP.add, OP.mult)
                        ht = tp.tile([H, BL], F32, tag=f"ht{d}",
                                     name=f"ht{d}")
                        nc.tensor.transpose(ht[:], hb[:], id16[:])
                        nc.vector.tensor_copy(hist_out[d][:, sl], ht[:])
                        cprev[d] = cnew

        scan_layer(0, h0T)
        h1T = [big.tile([H, NTB], F32R, tag="bigA", name="h1T0"),
               big.tile([H, NTB], F32R, tag="bigB", name="h1T1")]
        scan_layer(1, h1T)

        # ---------------- P3a: emissions ----------------
        emr = big.tile([K, NTB], F32, tag="h0T0")     # b-major: col=b*nt+t
        expem = big.tile([K, NTB], F32, tag="h0T1")   # t-major: col=t*BL+b
        with tc.tile_pool(name="ep", bufs=2, space="PSUM") as ep:
            ECH = 512 if NTB % 512 == 0 else NTB
            etch = ECH // BL                          # t per chunk
            for c in range(NTB // ECH):
                pe = ep.tile([K, ECH], F32)
                sl = slice(c * ECH, (c + 1) * ECH)
                _mm(nc, pe[:], wout_sb[0][:], h1T[0][:, sl], True, False)
                _mm(nc, pe[:], wout_sb[1][:], h1T[1][:, sl], False, True)
                # write em (+bout) b-major via strided AP
                pe3 = pe.rearrange("p (t b) -> p t b", b=BL)
                emr3 = emr.rearrange("p (b t) -> p b t", b=BL)[
                    :, :, c * etch:(c + 1) * etch].rearrange("p b t -> p t b")
                nc.scalar.activation(emr3, pe3, AF.Identity, bias=bout_sb[:])
        # exp(em) in t-major layout
        emr_tm = emr.rearrange("p (b t) -> p t b", b=BL)
        expem3 = expem.rearrange("p (t b) -> p t b", b=BL)
        nc.scalar.activation(expem3, emr_tm, AF.Exp)

        # ---------------- P3b: CRF forward (denominator) ----------------
        with tc.tile_pool(name="cp", bufs=1, space="PSUM") as cp, \
             tc.tile_pool(name="sp", bufs=1, space="PSUM") as sp, \
             tc.tile_pool(name="npp", bufs=2, space="PSUM") as npp:
            # two independent half-batch chains interleave to hide latency
            NH = 2
            HB = BL // NH
            aps, logaccs, pendings = [], [], []
            for hh in range(NH):
                hs = slice(hh * HB, (hh + 1) * HB)
                a0 = stp.tile([K, HB], F32, tag=f"alpha{hh}", name=f"a0_{hh}")
                nc.vector.tensor_tensor(
                    a0[:], expem[:, hs],
                    expstart[:, 0:1].to_broadcast([K, HB]), OP.mult)
                la0 = stp.tile([1, HB], F32, tag=f"logacc{hh}",
                               name=f"la0_{hh}")
                nc.vector.memset(la0[:], 0.0)
                aps.append(a0)
                logaccs.append(la0)
                pendings.append(None)
            for t_ in range(1, nt):
                for hh in range(NH):
                    hs = slice(t_ * BL + hh * HB, t_ * BL + (hh + 1) * HB)
                    pa = cp.tile([K, HB], F32, tag=f"pa{hh}", name=f"pa{hh}")
                    _mm(nc, pa[:], eexp[:], aps[hh][:], True, True,
                        fast=False)
                    an = stp.tile([K, HB], F32, tag=f"alpha{hh}",
                                  name=f"an{hh}")
                    nc.vector.tensor_tensor(an[:], pa[:], expem[:, hs],
                                            OP.mult)
                    aps[hh] = an
                    if pendings[hh] is not None and t_ >= pendings[hh][1]:
                        asc = stp.tile([K, HB], F32, tag=f"alpha{hh}",
                                       name=f"as{hh}")
                        nc.vector.tensor_tensor(
                            asc[:], aps[hh][:], pendings[hh][0][:], OP.mult)
                        aps[hh] = asc
                        pendings[hh] = None
                    if t_ % RESCALE == 0 and t_ + 2 < nt:
                        ps = sp.tile([K, HB], F32, tag=f"ps{hh}",
                                     name=f"ps{hh}")
                        _mm(nc, ps[:], ones2020[:], aps[hh][:], True, True,
                            fast=False)
                        sinv = work.tile([K, HB], F32, tag=f"sinv{hh}",
                                         name=f"sinv{hh}")
                        nc.vector.reciprocal(sinv[:], ps[:])
                        lt = work.tile([1, HB], F32, tag=f"lt{hh}",
                                       name=f"lt{hh}")
                        nc.scalar.activation(lt[:], ps[0:1, :], AF.Ln)
                        la = stp.tile([1, HB], F32, tag=f"logacc{hh}",
                                      name=f"lan{hh}")
                        nc.vector.tensor_tensor(la[:], logaccs[hh][:], lt[:],
                                                OP.add)
                        logaccs[hh] = la
                        pendings[hh] = (sinv, t_ + 2)
            logz = work.tile([1, BL], F32, tag="logz")
            for hh in range(NH):
                if pendings[hh] is not None:
                    asc = stp.tile([K, HB], F32, tag=f"alpha{hh}",
                                   name=f"af{hh}")
                    nc.vector.tensor_tensor(asc[:], aps[hh][:],
                                            pendings[hh][0][:], OP.mult)
                    aps[hh] = asc
                aend = work.tile([K, HB], F32, tag=f"aend{hh}",
                                 name=f"aend{hh}")
                nc.vector.tensor_tensor(
                    aend[:], aps[hh][:],
                    expend[:, 0:1].to_broadcast([K, HB]), OP.mult)
                psf = sp.tile([K, HB], F32, tag=f"ps{hh}", name=f"psf{hh}")
                _mm(nc, psf[:], ones2020[:], aend[:], True, True, fast=False)
                lnf = work.tile([1, HB], F32, tag=f"lnf{hh}",
                                name=f"lnf{hh}")
                nc.scalar.activation(lnf[:], psf[0:1, :], AF.Ln)
                nc.vector.tensor_tensor(
                    logz[:, hh * HB:(hh + 1) * HB], lnf[:], logaccs[hh][:],
                    OP.add)
            nc.sync.dma_start(out=outm[1:2, :], in_=logz[:])

            # ---------------- P3c: numerator (path score) ----------------
            tags_rep = big.tile([K, NTB], F32, tag="bigA", name="tags_rep")
            nc.sync.dma_start(out=tags_rep[:],
                              in_=tagsf[0:1, :].to_broadcast([K, NTB]))
            scol = stp.tile([K, BL], F32, tag="scol")
            spl = stp.tile([K, BL], F32, tag="spl")
            for b in range(BL):
                base = b * nt
                ohb = work.tile([K, nt], F32, tag="ohb")
                nc.vector.tensor_tensor(
                    ohb[:], iota20f[:, 0:1].to_broadcast([K, nt]),
                    tags_rep[:, base:base + nt], OP.is_equal)
                s1 = npp.tile([K, nt - 1], F32)
                _mm(nc, s1[:], trans_sb[:], ohb[:, 0:nt - 1], True, True)
                qa = work.tile([K, nt - 1], F32, tag="qa")
                nc.vector.tensor_tensor(
                    qa[:], s1[:], emr[:, base + 1:base + nt], OP.add)
                dump = work.tile([K, nt - 1], F32, tag="dump")
                nc.vector.scalar_tensor_tensor(
                    dump[:], qa[:], 0.0, ohb[:, 1:nt],
                    OP.add, OP.mult, accum_out=scol[:, b:b + 1])
                t0 = work.tile([K, 1], F32, tag="t0")
                nc.vector.scalar_tensor_tensor(
                    t0[:], emr[:, base:base + 1], start_sb[:, 0:1],
                    ohb[:, 0:1], OP.add, OP.mult)
                te = work.tile([K, 1], F32, tag="te")
                nc.vector.tensor_tensor(
                    te[:], ohb[:, nt - 1:nt], end_sb[:, 0:1], OP.mult)
                nc.vector.tensor_tensor(spl[:, b:b + 1], t0[:], te[:], OP.add)
            psc = sp.tile([K, BL], F32, tag="psc")
            _mm(nc, psc[:], ones2020[:], scol[:], True, False, fast=False)
            _mm(nc, psc[:], ones2020[:], spl[:], False, True, fast=False)
            score = work.tile([1, BL], F32, tag="score")
            nc.vector.tensor_copy(score[:], psc[0:1, :])
            nc.sync.dma_start(out=outm[0:1, :], in_=score[:])

    nc.compile()
    return nc


# ---------------------------------------------------------------------------
# Host side
# ---------------------------------------------------------------------------
_CACHE = {}


def _get_nc(nt):
    if nt not in _CACHE:
        _CACHE[nt] = build(nt)
    return _CACHE[nt]


def prep_inputs(sentences, tags, embed, Wih0, Whh0, b0, Wih1, Whh1, b1,
                Wout, bout, trans, start, end, nt=T):
    """Host-side marshalling: weight transposes + power-of-2 gate rescales."""
    f32 = np.float32
    sc = np.ones((G4, 1), f32)
    sc[2 * H:3 * H] = 2.0           # g rows: tanh-via-sigmoid needs 2x

    def stack2(w, s):
        return np.stack([np.ascontiguousarray((w[d] * s).T.astype(f32))
                         for d in range(2)])

    wihT0 = stack2(Wih0, sc)                    # [2,128,512] (transposed)
    whhT0 = stack2(Whh0, 2.0 * sc)
    b0v = np.stack([(b0[d][None, :] * sc[:, 0][None, :]).astype(f32)
                    for d in range(2)])
    wih1T_full = stack2(Wih1, 2.0 * sc)         # [2,256,512]
    wih1T = wih1T_full.reshape(2, 2, H, G4)
    whh1T = stack2(Whh1, 2.0 * sc)
    b1v = np.stack([(b1[d][None, :] * sc[:, 0][None, :]).astype(f32)
                    for d in range(2)])
    woutT = np.stack([np.ascontiguousarray((2.0 * Wout[:, :H]).T.astype(f32)),
                      np.ascontiguousarray((2.0 * Wout[:, H:]).T.astype(f32))])
    shared = dict(
        embedb=np.ascontiguousarray(embed.astype(ml_dtypes.bfloat16)),
        wihT0=wihT0, whhT0=whhT0, b0v=b0v, wih1T=wih1T, whh1T=whh1T, b1v=b1v,
        woutT=woutT, boutv=bout.reshape(K, 1).astype(f32),
        transm=trans.astype(f32), startv=start.reshape(K, 1).astype(f32),
        endv=end.reshape(K, 1).astype(f32),
    )
    in_maps = []
    for c in range(NCORES):
        bsl = slice(c * BL, (c + 1) * BL)
        m = dict(shared)
        m["toks16"] = np.ascontiguousarray(
            sentences[bsl, :nt].astype(np.int16))
        m["tagsf"] = np.ascontiguousarray(
            tags[bsl, :nt].astype(f32).reshape(1, BL * nt))
        in_maps.append(m)
    return in_maps


def run(inputs_np, nt=T, trace=False):
    nc = _get_nc(nt)
    in_maps = prep_inputs(
        inputs_np["sentences"], inputs_np["tags"], inputs_np["embed"],
        inputs_np["Wih0"], inputs_np["Whh0"], inputs_np["b0"],
        inputs_np["Wih1"], inputs_np["Whh1"], inputs_np["b1"],
        inputs_np["Wout"], inputs_np["bout"], inputs_np["trans"],
        inputs_np["start"], inputs_np["end"], nt=nt)
    res = run_bass_kernel_spmd(nc, in_maps, core_ids=list(range(NCORES)),
                               trace=trace)
    score = np.concatenate([res.results[c]["outm"][0] for c in range(NCORES)])
    logz = np.concatenate([res.results[c]["outm"][1] for c in range(NCORES)])
    loss = -np.mean(score - logz)
    return np.float32(loss), res


def kernel(**inputs):
    inputs_np = {k: np.asarray(v) for k, v in inputs.items()}
    loss, _ = run(inputs_np, nt=T)
    return np.asarray(loss, dtype=np.float32)



# revision 2
# speedup vs baseline: 1.0042x; 1.0042x over previous
"""BiLSTM-CRF loss kernel for 8x Trainium2 NeuronCores (Bass/Tile).

Feature-on-partition redesign of the baseline:
 - Gate math runs as [H=128, BL=16] tiles (all 128 lanes active) instead of
   [16, 4H] (16 lanes): DVE/Act ops are ~8x cheaper.
 - Recurrence/input projections are per-gate [128,16]-out bf16 matmuls
   accumulated in PSUM (bias via 1-row matmul), so no transpose is needed:
   h is produced in exactly the layout the next matmul consumes.
 - h history is stored bf16 and feeds layer1 / emissions matmuls directly.
 - CRF forward runs two half-batch chains; chain 1's elementwise multiply
   goes to gpsimd (Pool) to take it off DVE.

Math identical to baseline:
 - tanh(x) = 2*sigmoid(2x)-1; h~=h/2, c~=c/2 with the factors of 2 folded
   into weights host-side.  Per tick: s = sigmoid(z'), u=(s_g-.5)*s_i,
   c~ = s_f*c~ + u, h~ = (sigmoid(4c~)-.5)*s_o.
 - CRF forward in exp space with partition-sum rescale every RESCALE steps.
"""

import sys

sys.path.insert(0, "/opt/trn_rl_repo")

import contextlib

import numpy as np
import ml_dtypes

import concourse.bass as bass
import concourse.tile as tile
from concourse import bacc, mybir
from concourse.bass_utils import run_bass_kernel_spmd

F32 = mybir.dt.float32
BF16 = mybir.dt.bfloat16
I16 = mybir.dt.int16
AF = mybir.ActivationFunctionType
OP = mybir.AluOpType

NCORES = 8
B, T, E, H, K, V = 128, 512, 128, 128, 20, 30000
G4 = 4 * H
BL = B // NCORES    # 16 sentences per core
RESCALE = 8


def build(nt=T):
    nc = bacc.Bacc("TRN2", target_bir_lowering=False, debug=False,
                   num_devices=NCORES)
    NTB = nt * BL

    # ---- DRAM I/O ----
    embedb = nc.dram_tensor("embedb", [V, E], BF16, kind="ExternalInput")
    toks16 = nc.dram_tensor("toks16", [BL, nt], I16, kind="ExternalInput")
    tagsf = nc.dram_tensor("tagsf", [1, NTB], F32, kind="ExternalInput")
    # per (dir, gate) transposed weights, bf16
    wih0T = nc.dram_tensor("wih0T", [2, 4, E, H], BF16, kind="ExternalInput")
    whh0T = nc.dram_tensor("whh0T", [2, 4, H, H], BF16, kind="ExternalInput")
    b0r = nc.dram_tensor("b0r", [2, 4, 1, H], BF16, kind="ExternalInput")
    wih1aT = nc.dram_tensor("wih1aT", [2, 4, H, H], BF16, kind="ExternalInput")
    wih1bT = nc.dram_tensor("wih1bT", [2, 4, H, H], BF16, kind="ExternalInput")
    whh1T = nc.dram_tensor("whh1T", [2, 4, H, H], BF16, kind="ExternalInput")
    b1r = nc.dram_tensor("b1r", [2, 4, 1, H], BF16, kind="ExternalInput")
    woutT = nc.dram_tensor("woutT", [2, H, K], BF16, kind="ExternalInput")
    boutv = nc.dram_tensor("boutv", [K, 1], F32, kind="ExternalInput")
    transm = nc.dram_tensor("transm", [K, K], F32, kind="ExternalInput")
    startv = nc.dram_tensor("startv", [K, 1], F32, kind="ExternalInput")
    endv = nc.dram_tensor("endv", [K, 1], F32, kind="ExternalInput")
    outm = nc.dram_tensor("outm", [2, BL], F32, kind="ExternalOutput")

    with tile.TileContext(nc) as tc, contextlib.ExitStack() as ctx:
        big = ctx.enter_context(tc.tile_pool(name="big", bufs=1))
        wp = ctx.enter_context(tc.tile_pool(name="wp", bufs=1))
        work = ctx.enter_context(tc.tile_pool(name="work", bufs=3))
        stp = ctx.enter_context(tc.tile_pool(name="stp", bufs=2))

        # ---------------- P0: constants, weights, gather ----------------
        idx = wp.tile([128, nt], I16, tag="idx")
        nc.gpsimd.memset(idx[:], 0)
        nc.sync.dma_start(out=idx[0:BL, :], in_=toks16[:, :])

        def load_w(name, dram_ap, shape, dt=F32):
            t = wp.tile(shape, dt, tag=name)
            nc.sync.dma_start(out=t[:], in_=dram_ap)
            return t

        wih0_sb = [[load_w(f"wih0_{d}{g}", wih0T[d, g], [E, H], BF16)
                    for g in range(4)] for d in range(2)]
        whh0_sb = [[load_w(f"whh0_{d}{g}", whh0T[d, g], [H, H], BF16)
                    for g in range(4)] for d in range(2)]
        b0_sb = [[load_w(f"b0_{d}{g}", b0r[d, g], [1, H], BF16)
                  for g in range(4)] for d in range(2)]
        wih1a_sb = [[load_w(f"wih1a_{d}{g}", wih1aT[d, g], [H, H], BF16)
                     for g in range(4)] for d in range(2)]
        wih1b_sb = [[load_w(f"wih1b_{d}{g}", wih1bT[d, g], [H, H], BF16)
                     for g in range(4)] for d in range(2)]
        whh1_sb = [[load_w(f"whh1_{d}{g}", whh1T[d, g], [H, H], BF16)
                    for g in range(4)] for d in range(2)]
        b1_sb = [[load_w(f"b1_{d}{g}", b1r[d, g], [1, H], BF16)
                  for g in range(4)] for d in range(2)]
        wout_sb = [load_w(f"wout_{d}", woutT[d], [H, K], BF16)
                   for d in range(2)]
        bout_sb = load_w("bout", boutv[:, :], [K, 1])
        trans_sb = load_w("trans", transm[:, :], [K, K])
        start_sb = load_w("start", startv[:, :], [K, 1])
        end_sb = load_w("end", endv[:, :], [K, 1])

        ones16 = wp.tile([1, BL], BF16, tag="ones16")
        nc.vector.memset(ones16[:], 1.0)
        ones2020 = wp.tile([K, K], F32, tag="ones2020")
        nc.vector.memset(ones2020[:], 1.0)
        iota20 = wp.tile([K, 1], mybir.dt.int32, tag="iota20i")
        nc.gpsimd.iota(iota20[:], pattern=[[0, 1]], base=0,
                       channel_multiplier=1)
        iota20f = wp.tile([K, 1], F32, tag="iota20f")
        nc.vector.tensor_copy(iota20f[:], iota20[:])
        eexp = wp.tile([K, K], F32, tag="eexp")
        nc.scalar.activation(eexp[:], trans_sb[:], AF.Exp)
        expstart = wp.tile([K, 1], F32, tag="expstart")
        nc.scalar.activation(expstart[:], start_sb[:], AF.Exp)
        expend = wp.tile([K, 1], F32, tag="expend")
        nc.scalar.activation(expend[:], end_sb[:], AF.Exp)

        # Embedding gather (+transpose): xg[128_E, NTB] bf16, col = t*BL+b.
        # Chunks issued outside-in (0, last, 1, last-1, ...) so the fwd and
        # bwd scans can start as soon as the first pair lands.
        xg = big.tile([128, 1, NTB], BF16, tag="bigX")
        GCH = 256  # idxs per gather (SWDGE descriptor-ring limit)
        nch = max(1, NTB // GCH)
        order = []
        for i in range((nch + 1) // 2):
            order.append(i)
            if nch - 1 - i != i:
                order.append(nch - 1 - i)
        for g in order:
            cw = min(GCH, NTB)
            nc.gpsimd.dma_gather(
                xg[:, :, g * cw:(g + 1) * cw], embedb[:, :],
                idx[:, g * (cw // 16):(g + 1) * (cw // 16)],
                cw, cw, E, transpose=True)

        # Histories (feature-on-partition, bf16, col = t*BL + b)
        h0T = [big.tile([H, NTB], BF16, tag=f"h0T{d}", name=f"h0T{d}")
               for d in range(2)]
        h1T = [big.tile([H, NTB], BF16, tag=f"h1T{d}", name=f"h1T{d}")
               for d in range(2)]

        # ---------------- P1 / P2: the two BiLSTM layers ----------------
        def scan_layer(layer, hist_out):
            xin = (None, wih0_sb, None) if layer == 0 else \
                  (h0T, wih1a_sb, wih1b_sb)
            whh_sb = whh0_sb if layer == 0 else whh1_sb
            bb_sb = b0_sb if layer == 0 else b1_sb
            with tc.tile_pool(name=f"zp{layer}", bufs=2, space="PSUM") as zp:
                cprev = []
                for d in range(2):
                    cp0 = stp.tile([H, BL], F32, tag=f"c{layer}{d}",
                                   name=f"c{layer}{d}")
                    nc.vector.memset(cp0[:], 0.0)
                    cprev.append(cp0)
                for n in range(nt):
                    tt_ = [n, nt - 1 - n]
                    zs = []
                    # input-side matmuls for both dirs first (independent),
                    # then the h-recurrence matmuls (the serial dependency)
                    for d in range(2):
                        t_ = tt_[d]
                        sl = slice(t_ * BL, (t_ + 1) * BL)
                        z = zp.tile([H, 4 * BL], F32, tag=f"z{d}",
                                    name=f"z{d}")
                        zs.append(z)
                        for g in range(4):
                            gsl = slice(g * BL, (g + 1) * BL)
                            nc.tensor.matmul(z[:, gsl], bb_sb[d][g][:],
                                             ones16[:], start=True,
                                             stop=False)
                            if layer == 0:
                                nc.tensor.matmul(z[:, gsl], wih0_sb[d][g][:],
                                                 xg[:, 0, sl],
                                                 start=False, stop=(n == 0))
                            else:
                                nc.tensor.matmul(z[:, gsl], wih1a_sb[d][g][:],
                                                 h0T[0][:, sl],
                                                 start=False, stop=False)
                                nc.tensor.matmul(z[:, gsl], wih1b_sb[d][g][:],
                                                 h0T[1][:, sl],
                                                 start=False, stop=(n == 0))
                    for d in range(2):
                        if n > 0:
                            tprev = tt_[d] + (-1 if d == 0 else 1)
                            psl = slice(tprev * BL, (tprev + 1) * BL)
                            for g in range(4):
                                gsl = slice(g * BL, (g + 1) * BL)
                                nc.tensor.matmul(zs[d][:, gsl],
                                                 whh_sb[d][g][:],
                                                 hist_out[d][:, psl],
                                                 start=False, stop=True)
                    # stage-major emission keeps the two direction chains
                    # decoupled on each in-order engine queue: Act runs
                    # [sigA sigB scA scB], DVE runs [uA fcA cA uB fcB cB
                    # hA hB] so neither chain's stall blocks the other.
                    ss, cnews, scs = [], [], []
                    for d in range(2):
                        s = work.tile([H, 4 * BL], F32, tag=f"s{d}",
                                      name=f"s{d}")
                        nc.scalar.activation(s[:], zs[d][:], AF.Sigmoid)
                        ss.append(s)
                    for d in range(2):
                        s = ss[d]
                        u = work.tile([H, BL], F32, tag=f"u{d}", name=f"u{d}")
                        nc.vector.scalar_tensor_tensor(
                            u[:], s[:, 2 * BL:3 * BL], -0.5,
                            s[:, 0 * BL:1 * BL], OP.add, OP.mult)
                        fc = work.tile([H, BL], F32, tag=f"fc{d}",
                                       name=f"fc{d}")
                        nc.vector.tensor_tensor(fc[:], s[:, 1 * BL:2 * BL],
                                                cprev[d][:], OP.mult)
                        cnew = stp.tile([H, BL], F32, tag=f"c{layer}{d}",
                                        name=f"cn{layer}{d}")
                        nc.vector.tensor_tensor(cnew[:], fc[:], u[:], OP.add)
                        cnews.append(cnew)
                    for d in range(2):
                        sc = work.tile([H, BL], F32, tag=f"sc{d}",
                                       name=f"sc{d}")
                        nc.scalar.activation(sc[:], cnews[d][:], AF.Sigmoid,
                                             scale=4.0)
                        scs.append(sc)
                    for d in range(2):
                        t_ = tt_[d]
                        sl = slice(t_ * BL, (t_ + 1) * BL)
                        nc.vector.scalar_tensor_tensor(
                            hist_out[d][:, sl], scs[d][:], -0.5,
                            ss[d][:, 3 * BL:4 * BL], OP.add, OP.mult)
                        cprev[d] = cnews[d]

        scan_layer(0, h0T)
        scan_layer(1, h1T)

        # ---------------- P3a: emissions ----------------
        emr = big.tile([K, NTB], F32, tag="emr")     # b-major: col=b*nt+t
        expem = big.tile([K, NTB], F32, tag="expem")  # t-major: col=t*BL+b
        with tc.tile_pool(name="ep", bufs=2, space="PSUM") as ep:
            ECH = 512 if NTB % 512 == 0 else NTB
            etch = ECH // BL
            for c in range(NTB // ECH):
                pe = ep.tile([K, ECH], F32)
                sl = slice(c * ECH, (c + 1) * ECH)
                nc.tensor.matmul(pe[:], wout_sb[0][:], h1T[0][:, sl],
                                 start=True, stop=False)
                nc.tensor.matmul(pe[:], wout_sb[1][:], h1T[1][:, sl],
                                 start=False, stop=True)
                pe3 = pe.rearrange("p (t b) -> p t b", b=BL)
                emr3 = emr.rearrange("p (b t) -> p b t", b=BL)[
                    :, :, c * etch:(c + 1) * etch].rearrange("p b t -> p t b")
                nc.scalar.activation(emr3, pe3, AF.Identity, bias=bout_sb[:])
                # exp of this t-chunk immediately so the CRF can start
                # before the full emissions sweep finishes
                expem3 = expem.rearrange("p (t b) -> p t b", b=BL)[
                    :, c * etch:(c + 1) * etch]
                nc.scalar.activation(expem3, pe3, AF.Exp, bias=bout_sb[:])

        # ---------------- P3b: CRF forward + numerator, interleaved -------
        # Single full-batch [K, BL] alpha chain (parallel lanes don't cut
        # wall time; one lane halves the instruction count).  The numerator's
        # per-sentence blocks are emitted every nt//BL steps so their DVE/PE
        # work fills the chain's latency gaps.
        with tc.tile_pool(name="cp", bufs=2, space="PSUM") as cp, \
             tc.tile_pool(name="sp", bufs=1, space="PSUM") as sp, \
             tc.tile_pool(name="npp", bufs=2, space="PSUM") as npp:
            scol = stp.tile([K, BL], F32, tag="scol")
            spl = stp.tile([K, BL], F32, tag="spl")

            def numer_block(b):
                base = b * nt
                tgr = work.tile([K, nt], F32, tag="tgr", name=f"tgr{b}")
                nc.sync.dma_start(
                    out=tgr[:],
                    in_=tagsf[0:1, base:base + nt].to_broadcast([K, nt]))
                ohb = work.tile([K, nt], F32, tag="ohb")
                nc.vector.tensor_tensor(
                    ohb[:], iota20f[:, 0:1].to_broadcast([K, nt]),
                    tgr[:], OP.is_equal)
                s1 = npp.tile([K, nt - 1], F32)
                nc.tensor.matmul(s1[:], trans_sb[:], ohb[:, 0:nt - 1],
                                 start=True, stop=True)
                qa = work.tile([K, nt - 1], F32, tag="qa")
                nc.vector.tensor_tensor(
                    qa[:], s1[:], emr[:, base + 1:base + nt], OP.add)
                dump = work.tile([K, nt - 1], F32, tag="dump")
                nc.vector.scalar_tensor_tensor(
                    dump[:], qa[:], 0.0, ohb[:, 1:nt],
                    OP.add, OP.mult, accum_out=scol[:, b:b + 1])
                t0 = work.tile([K, 1], F32, tag="t0")
                nc.vector.scalar_tensor_tensor(
                    t0[:], emr[:, base:base + 1], start_sb[:, 0:1],
                    ohb[:, 0:1], OP.add, OP.mult)
                te = work.tile([K, 1], F32, tag="te")
                nc.vector.tensor_tensor(
                    te[:], ohb[:, nt - 1:nt], end_sb[:, 0:1], OP.mult)
                nc.vector.tensor_tensor(spl[:, b:b + 1], t0[:], te[:], OP.add)

            ap = stp.tile([K, BL], F32, tag="alpha", name="a0")
            nc.vector.tensor_tensor(
                ap[:], expem[:, 0:BL],
                expstart[:, 0:1].to_broadcast([K, BL]), OP.mult)
            logacc = stp.tile([1, BL], F32, tag="logacc", name="la0")
            nc.vector.memset(logacc[:], 0.0)
            pending = None
            nstep = max(1, (nt - 1) // BL)
            nb_done = 0
            for t_ in range(1, nt):
                hs = slice(t_ * BL, (t_ + 1) * BL)
                pa = cp.tile([K, BL], F32, tag="pa", name="pa")
                nc.tensor.matmul(pa[:], eexp[:], ap[:],
                                 start=True, stop=True)
                an = stp.tile([K, BL], F32, tag="alpha", name="an")
                nc.vector.tensor_tensor(an[:], pa[:], expem[:, hs], OP.mult)
                ap = an
                if pending is not None and t_ >= pending[1]:
                    asc = stp.tile([K, BL], F32, tag="alpha", name="as")
                    nc.gpsimd.tensor_tensor(
                        asc[:], ap[:], pending[0][:], OP.mult)
                    ap = asc
                    pending = None
                if t_ % RESCALE == 0 and t_ + 2 < nt:
                    ps = sp.tile([K, BL], F32, tag="ps", name="ps")
                    nc.tensor.matmul(ps[:], ones2020[:], ap[:],
                                     start=True, stop=True)
                    sinv = work.tile([K, BL], F32, tag="sinv", name="sinv")
                    nc.vector.reciprocal(sinv[:], ps[:])
                    lt = work.tile([1, BL], F32, tag="lt", name="lt")
                    nc.scalar.activation(lt[:], ps[0:1, :], AF.Ln)
                    la = stp.tile([1, BL], F32, tag="logacc", name="lan")
                    nc.vector.tensor_tensor(la[:], logacc[:], lt[:], OP.add)
                    logacc = la
                    pending = (sinv, t_ + 2)
                if t_ % nstep == 0 and nb_done < BL:
                    numer_block(nb_done)
                    nb_done += 1
            logz = work.tile([1, BL], F32, tag="logz")
            if pending is not None:
                asc = stp.tile([K, BL], F32, tag="alpha", name="af")
                nc.vector.tensor_tensor(asc[:], ap[:], pending[0][:],
                                        OP.mult)
                ap = asc
            aend = work.tile([K, BL], F32, tag="aend")
            nc.vector.tensor_tensor(
                aend[:], ap[:], expend[:, 0:1].to_broadcast([K, BL]),
                OP.mult)
            psf = sp.tile([K, BL], F32, tag="ps", name="psf")
            nc.tensor.matmul(psf[:], ones2020[:], aend[:],
                             start=True, stop=True)
            lnf = work.tile([1, BL], F32, tag="lnf")
            nc.scalar.activation(lnf[:], psf[0:1, :], AF.Ln)
            nc.vector.tensor_tensor(logz[:], lnf[:], logacc[:], OP.add)
            nc.sync.dma_start(out=outm[1:2, :], in_=logz[:])
            for b in range(nb_done, BL):
                numer_block(b)
            psc = sp.tile([K, BL], F32, tag="psc")
            nc.tensor.matmul(psc[:], ones2020[:], scol[:],
                             start=True, stop=False)
            nc.tensor.matmul(psc[:], ones2020[:], spl[:],
                             start=False, stop=True)
            score = work.tile([1, BL], F32, tag="score")
            nc.vector.tensor_copy(score[:], psc[0:1, :])
            nc.sync.dma_start(out=outm[0:1, :], in_=score[:])

    nc.compile()
    return nc


# ---------------------------------------------------------------------------
# Host side
# ---------------------------------------------------------------------------
_CACHE = {}


def _get_nc(nt):
    if nt not in _CACHE:
        _CACHE[nt] = build(nt)
    return _CACHE[nt]


def prep_inputs(sentences, tags, embed, Wih0, Whh0, b0, Wih1, Whh1, b1,
                Wout, bout, trans, start, end, nt=T):
    """Host-side marshalling: per-gate transposed bf16 weights with the
    tanh-via-sigmoid (x2 on g rows) and h~=h/2 (x2 on all recurrent/input
    uses of h) scalings folded in."""
    f32 = np.float32
    bf16 = ml_dtypes.bfloat16
    gs = np.array([1.0, 1.0, 2.0, 1.0], f32)  # i,f,g,o row scales

    def per_gate_T(w, hscale):
        # w: [2, 4H, D] -> [2, 4, D, H] bf16, gate g scaled by gs[g]*hscale
        out = np.empty((2, 4, w.shape[2], H), bf16)
        for d in range(2):
            for g in range(4):
                out[d, g] = (w[d][g * H:(g + 1) * H, :]
                             * (gs[g] * hscale)).T.astype(bf16)
        return np.ascontiguousarray(out)

    def per_gate_b(bv):
        out = np.empty((2, 4, 1, H), bf16)
        for d in range(2):
            for g in range(4):
                out[d, g, 0] = (bv[d][g * H:(g + 1) * H] * gs[g]).astype(bf16)
        return np.ascontiguousarray(out)

    wih0T = per_gate_T(Wih0, 1.0)                    # [2,4,E,H]
    whh0T = per_gate_T(Whh0, 2.0)
    wih1T_full = per_gate_T(Wih1, 2.0)               # [2,4,2H,H]
    wih1aT = np.ascontiguousarray(wih1T_full[:, :, :H, :])
    wih1bT = np.ascontiguousarray(wih1T_full[:, :, H:, :])
    whh1T = per_gate_T(Whh1, 2.0)
    woutT = np.stack([
        np.ascontiguousarray((2.0 * Wout[:, :H]).T.astype(bf16)),
        np.ascontiguousarray((2.0 * Wout[:, H:]).T.astype(bf16))])
    shared = dict(
        embedb=np.ascontiguousarray(embed.astype(bf16)),
        wih0T=wih0T, whh0T=whh0T, b0r=per_gate_b(b0),
        wih1aT=wih1aT, wih1bT=wih1bT, whh1T=whh1T, b1r=per_gate_b(b1),
        woutT=woutT, boutv=bout.reshape(K, 1).astype(f32),
        transm=trans.astype(f32), startv=start.reshape(K, 1).astype(f32),
        endv=end.reshape(K, 1).astype(f32),
    )
    in_maps = []
    for c in range(NCORES):
        bsl = slice(c * BL, (c + 1) * BL)
        m = dict(shared)
        m["toks16"] = np.ascontiguousarray(
            sentences[bsl, :nt].astype(np.int16))
        m["tagsf"] = np.ascontiguousarray(
            tags[bsl, :nt].astype(f32).reshape(1, BL * nt))
        in_maps.append(m)
    return in_maps


def run(inputs_np, nt=T, trace=False):
    nc = _get_nc(nt)
    in_maps = prep_inputs(
        inputs_np["sentences"], inputs_np["tags"], inputs_np["embed"],
        inputs_np["Wih0"], inputs_np["Whh0"], inputs_np["b0"],
        inputs_np["Wih1"], inputs_np["Whh1"], inputs_np["b1"],
        inputs_np["Wout"], inputs_np["bout"], inputs_np["trans"],
        inputs_np["start"], inputs_np["end"], nt=nt)
    res = run_bass_kernel_spmd(nc, in_maps, core_ids=list(range(NCORES)),
                               trace=trace)
    score = np.concatenate([res.results[c]["outm"][0] for c in range(NCORES)])
    logz = np.concatenate([res.results[c]["outm"][1] for c in range(NCORES)])
    loss = -np.mean(score - logz)
    return np.float32(loss), res


def kernel(**inputs):
    inputs_np = {k: np.asarray(v) for k, v in inputs.items()}
    loss, _ = run(inputs_np, nt=T)
    return np.asarray(loss, dtype=np.float32)
